# revision 13
# baseline (speedup 1.0000x reference)
"""HAN (2-layer heterogeneous GAT) on 8 Trainium2 NeuronCores (Bass/Tile).

v4: two launches total.  Node tables live in block layout (node id ->
blk*SLB + local); every table row is 256B.  k1: AllGather x slices ->
project (reading node-major tables through transpose-DMA) -> edge softmax
(src rows carry h only, es computed on-chip; dst rows carry
[scores f32|dmod f32]; one-hot scatter built on-chip via is_equal(iota,
dmod); gathers spread over 4 SWDGE queues) -> per-slice outputs + pw1
partial.  k2: AllGather layer-1 outputs + pw1, semantic-attention beta
on-chip, layer 2 (same structure), AllReduce pw2, final beta combine.

Compile + staging are untimed; EXEC_NS reports the mean per-execution wall
time of N pipelined repetitions of the k1->k2 chain.
"""
import numpy as np
import ml_dtypes

import concourse.bacc as bacc
import concourse.tile as tile
import concourse.mybir as mybir
from concourse import bass_utils  # noqa: F401

BF = ml_dtypes.bfloat16
N_A, N_P, E, NC = 50000, 100000, 800000, 8
SL_A, SL_P = N_A // NC, N_P // NC                # 6250, 12500
W_A, W_P = (SL_A + 127) // 128, (SL_P + 127) // 128  # 49, 98
PAD_A, PAD_P = W_A * 128, W_P * 128              # 6272, 12544 (= SLB)
NA_BLK, NP_BLK = NC * PAD_A, NC * PAD_P          # 50176, 100352
CHK = 32768
EPS = 1e-6
CT = 32                                          # tiles per device chunk

f32, bf16, i16 = mybir.dt.float32, mybir.dt.bfloat16, mybir.dt.int16
ADD, MULT, MAX = mybir.AluOpType.add, mybir.AluOpType.mult, mybir.AluOpType.max
EQ = mybir.AluOpType.is_equal
BYP = mybir.AluOpType.bypass
AF = mybir.ActivationFunctionType
GRP = [list(range(NC))]


# ---------------------------------------------------------------- host prep --
def pack16(idx):
    t = np.ascontiguousarray(idx.reshape(-1, 16).T.astype(np.int16))
    return np.tile(t, (8, 1))


def prep_type(src, dst, n_dst, src_sl, src_slb, n_win):
    """Uniform-schedule edge prep for one edge type across all 8 cores.

    src ids are remapped into block layout: id -> (id//src_sl)*src_slb +
    id%src_sl.  dst ids stay window-local within the owning core's slice."""
    sl = n_dst // NC
    n_src_rows = NC * src_slb
    n_chk = (n_src_rows + CHK - 1) // CHK
    K = n_chk * n_win
    sent = n_win * 128
    per = []
    for c in range(NC):
        m = (dst >= c * sl) & (dst < (c + 1) * sl)
        es = src[m].astype(np.int64)
        es = (es // src_sl) * src_slb + es % src_sl
        ed = (dst[m] - c * sl).astype(np.int64)
        key = (es // CHK) * n_win + (ed >> 7)
        o = np.argsort(key, kind="stable")
        per.append((es[o], ed[o], key[o]))
    cnts = np.stack([np.bincount(p[2], minlength=K) for p in per])
    T = (cnts.max(0) + 127) // 128
    keys = np.nonzero(T)[0]
    offs = np.zeros(K + 1, np.int64)
    offs[1:] = np.cumsum(T) * 128
    n_tiles = int(T.sum())
    npad = n_tiles * 128
    tw, tfirst, tlast, tcopy = [], [], [], []
    seen = set()
    for k in keys:
        w = int(k % n_win)
        nt = int(T[k])
        tw += [w] * nt
        tfirst += [True] + [False] * (nt - 1)
        tlast += [False] * (nt - 1) + [True]
        tcopy += [w not in seen] * nt
        seen.add(w)
    tchk = np.repeat(keys // n_win, T[keys])
    segs = []
    for c0 in range(0, n_tiles, CT):
        nt = min(CT, n_tiles - c0)
        cs, t = [], 0
        while t < nt:
            cb = int(tchk[c0 + t])
            t2 = t
            while t2 < nt and tchk[c0 + t2] == cb:
                t2 += 1
            cs.append((t, t2 - t, cb))
            t = t2
        segs.append(cs)
    s16, d16 = [], []
    for es, ed, key in per:
        sa = np.zeros(npad, np.int64)
        da = np.full(npad, sent, np.int64)
        st, cn = np.unique(key, return_index=True)
        cnt = np.diff(np.append(cn, len(key)))
        for k, s0, c_ in zip(st, cn, cnt):
            off = offs[k]
            sa[off:off + c_] = es[s0:s0 + c_] - (k // n_win) * CHK
            da[off:off + c_] = ed[s0:s0 + c_]
        s16.append(pack16(sa))
        d16.append(pack16(da))
    return dict(n_tiles=n_tiles, tw=tw, tfirst=tfirst, tlast=tlast,
                tcopy=tcopy, segs=segs, s16=s16, d16=d16)


def ablk(a, F):
    H = a.shape[0]
    o = np.zeros((F, H), np.float32)
    for h in range(H):
        o[h * 16:(h + 1) * 16, h] = a[h]
    return o


# ------------------------------------------------------------ device pieces --
def emit_edge_phase(nc, pool, psum, src_tbl, dst_tbl, s16d, d16d, meta,
                    F, H, accum, chunk_bases, iot, asb=None, so=0):
    """Edge softmax-accumulate for one edge type (see module docstring)."""
    NR = F + H
    n_tiles = meta["n_tiles"]
    tw, tf, tl, tc = meta["tw"], meta["tfirst"], meta["tlast"], meta["tcopy"]
    cur = [None]
    nrows = src_tbl.shape[0]
    for ci, c0 in enumerate(range(0, n_tiles, CT)):
        nt = min(CT, n_tiles - c0)
        si = pool.tile([128, nt * 8], i16, tag="si")
        di = pool.tile([128, nt * 8], i16, tag="di")
        nc.sync.dma_start(si[:], s16d[:, c0 * 8:(c0 + nt) * 8])
        nc.sync.dma_start(di[:], d16d[:, c0 * 8:(c0 + nt) * 8])
        G = pool.tile([128, nt, 128], bf16, tag="G")
        D = pool.tile([128, nt, 128], bf16, tag="D")
        for (t0, tn, cb) in meta["segs"][ci]:
            b = chunk_bases[cb]
            nc.gpsimd.dma_gather(
                out_ap=G[:, t0:t0 + tn, :],
                in_ap=src_tbl[b:min(b + CHK, nrows), :],
                idxs_ap=si[:, t0 * 8:(t0 + tn) * 8],
                num_idxs=tn * 128, num_idxs_reg=tn * 128, elem_size=128,
                single_packet=False, queue_num=ci % 2)
        nc.gpsimd.dma_gather(
            out_ap=D[:, 0:nt, :], in_ap=dst_tbl[:], idxs_ap=di[:],
            num_idxs=nt * 128, num_idxs_reg=nt * 128, elem_size=128,
            single_packet=False, queue_num=2 + ci % 2)
        Gf, Df = G[:].bitcast(f32), D[:].bitcast(f32)
        es = pool.tile([128, nt, H], f32, tag="es")
        if asb is not None:
            tmp = pool.tile([128, nt, 128], bf16, tag="tmp")
            nc.vector.tensor_tensor(
                tmp[:], G[:, 0:nt, :],
                asb[:, None, :].broadcast_to([128, nt, 128]), op=MULT)
            nc.vector.tensor_reduce(
                es[:], tmp[:].rearrange("p t (h d) -> p t h d", h=H),
                axis=mybir.AxisListType.X, op=ADD)
        else:
            nc.vector.tensor_copy(es[:], Gf[:, 0:nt, so:so + H])
        al = pool.tile([128, nt, H], f32, tag="al")
        nc.vector.tensor_tensor(al[:], es[:], Df[:, 0:nt, 0:H], op=ADD)
        lr = pool.tile([128, nt, H], f32, tag="lr")
        nc.vector.tensor_scalar(out=lr[:], in0=al[:], scalar1=0.2,
                                scalar2=None, op0=MULT)
        nc.vector.tensor_tensor(lr[:], lr[:], al[:], op=MAX)
        w = pool.tile([128, nt, H], f32, tag="w")
        nc.scalar.activation(w[:], lr[:], AF.Exp)
        M3 = pool.tile([128, nt, 128], bf16, tag="M3")
        nc.vector.tensor_tensor(
            M3[:], iot[:, None, :].broadcast_to([128, nt, 128]),
            Df[:, 0:nt, H:H + 1].broadcast_to([128, nt, 128]), op=EQ)
        VW = pool.tile([128, nt, NR], bf16, tag="VW")
        nc.vector.tensor_tensor(
            VW[:, :, 0:F].rearrange("p t (h d) -> p t h d", h=H),
            G[:, 0:nt, 0:F].rearrange("p t (h d) -> p t h d", h=H),
            w[:, :, :, None].broadcast_to([128, nt, H, 16]), op=MULT)
        nc.vector.tensor_copy(VW[:, :, F:NR], w[:])
        for t in range(nt):
            g = c0 + t
            if tf[g]:
                cur[0] = psum.tile([128, NR], f32, tag="eps", name="eps")
            nc.tensor.matmul(cur[0][:], M3[:, t, :], VW[:, t, :],
                             start=tf[g], stop=tl[g])
            if tl[g]:
                ws = accum[:, tw[g] * NR:(tw[g] + 1) * NR]
                if tc[g]:
                    nc.vector.tensor_copy(ws, cur[0][:])
                else:
                    nc.vector.tensor_tensor(ws, ws, cur[0][:], op=ADD)


def emit_normalize(nc, pool, accum, n_win, F, H, o_out, odt):
    NR = F + H
    a3 = accum.rearrange("p (w r) -> p w r", r=NR)
    o3 = o_out.rearrange("(w r) f -> r w f", r=128)
    for w0 in range(0, n_win, 4):
        nw = min(4, n_win - w0)
        rc = pool.tile([128, nw, H], f32, tag="rc")
        nc.vector.tensor_scalar(out=rc[:], in0=a3[:, w0:w0 + nw, F:NR],
                                scalar1=EPS, scalar2=None, op0=ADD)
        nc.vector.reciprocal(rc[:], rc[:])
        ot = pool.tile([128, nw, F], odt, tag="ot")
        nc.vector.tensor_tensor(
            ot[:].rearrange("p w (h d) -> p w h d", h=H),
            a3[:, w0:w0 + nw, 0:F].rearrange("p w (h d) -> p w h d", h=H),
            rc[:, :, :, None].broadcast_to([128, nw, H, 16]), op=MULT)
        nc.vector.tensor_scalar(out=ot[:], in0=ot[:], scalar1=0.0,
                                scalar2=None, op0=MAX)
        nc.sync.dma_start(o3[:, w0:w0 + nw, :], ot[:])


def emit_tanh_partial(nc, pool, psum, o_list, Wk_d, bk_d, q_d, F, ident,
                      n_pad_rows):
    Wkf = pool.tile([128, F], f32, tag="wkf")
    nc.sync.dma_start(Wkf[0:F, :], Wk_d[:])
    Wk = pool.tile([128, F], bf16, tag="wkb")
    nc.vector.tensor_copy(Wk[0:F, :], Wkf[0:F, :])
    bk = pool.tile([128, 1], f32, tag="bk")
    nc.sync.dma_start(bk[0:F, :], bk_d[:])
    qf = pool.tile([128, 1], f32, tag="qf")
    nc.sync.dma_start(qf[0:F, :], q_d[:])
    q = pool.tile([128, 1], bf16, tag="qb")
    nc.vector.tensor_copy(q[0:F, :], qf[0:F, :])
    tb = pool.tile([128, 1], f32, tag="tbk")
    nc.scalar.activation(tb[0:F, :], bk[0:F, :], AF.Tanh)
    corr_ps = psum.tile([1, 1], f32, tag="eps")
    nc.tensor.matmul(corr_ps[:], qf[0:F, 0:1], tb[0:F, :], start=True,
                     stop=True)
    corr = pool.tile([1, 1], f32, tag="corr")
    nc.vector.tensor_scalar(out=corr[:], in0=corr_ps[:],
                            scalar1=-float(n_pad_rows), scalar2=None,
                            op0=MULT)
    pw = pool.tile([1, 2], f32, tag="pw")
    for m, (o_d, npad, isf32) in enumerate(o_list):
        qacc = pool.tile([1, 128], f32, tag="qacc")
        nc.gpsimd.memset(qacc[:], 0.0)
        for t in range(npad // 128):
            oT = pool.tile([128, 128], bf16, tag="oT")
            if not isf32:
                nc.sync.dma_start_transpose(oT[0:F, :],
                                            o_d[t * 128:(t + 1) * 128, :])
            else:
                of = pool.tile([128, F], f32, tag="of")
                nc.sync.dma_start(of[:], o_d[t * 128:(t + 1) * 128, :])
                tp = psum.tile([128, 128], f32, tag="eps")
                nc.tensor.transpose(tp[0:F, :], of[:], ident[:])
                nc.vector.tensor_copy(oT[0:F, :], tp[0:F, :])
            ps2 = psum.tile([128, 128], f32, tag="eps")
            nc.tensor.matmul(ps2[0:F, :], Wk[0:F, :], oT[0:F, :],
                             start=True, stop=True)
            th = pool.tile([128, 128], bf16, tag="th")
            nc.scalar.activation(th[0:F, :], ps2[0:F, :], AF.Tanh,
                                 bias=bk[0:F, :])
            ps3 = psum.tile([1, 128], f32, tag="eps")
            nc.tensor.matmul(ps3[:], q[0:F, 0:1], th[0:F, :], start=True,
                             stop=True)
            nc.vector.tensor_tensor(qacc[:], qacc[:], ps3[:], op=ADD)
        red = pool.tile([1, 1], f32, tag="red")
        nc.vector.tensor_reduce(red[:], qacc[:], axis=mybir.AxisListType.X,
                                op=ADD)
        nc.vector.tensor_tensor(pw[0:1, m:m + 1], red[:], corr[:], op=ADD)
    return pw


def emit_beta_tail(nc, pool, psum, s, ones):
    """s: [1,2] SBUF tile of summed per-metapath scores (already /N)."""
    e = pool.tile([1, 2], f32, tag="pt3")
    nc.scalar.activation(e[:], s[:], AF.Exp)
    dn = pool.tile([1, 1], f32, tag="pt4")
    nc.vector.tensor_reduce(dn[:], e[:], axis=mybir.AxisListType.X, op=ADD)
    rcp = pool.tile([1, 1], f32, tag="pt5")
    nc.vector.reciprocal(rcp[:], dn[:])
    beta = pool.tile([1, 2], f32, tag="pt6")
    nc.vector.tensor_tensor(beta[:], e[:], rcp[:].broadcast_to([1, 2]),
                            op=MULT)
    cols = []
    for m in range(2):
        ps = psum.tile([128, 1], f32, tag="ps")
        nc.tensor.matmul(ps[:], ones[:], beta[0:1, m:m + 1], start=True,
                         stop=True)
        col = pool.tile([128, 1], f32, tag=f"bcol{m}")
        nc.vector.tensor_copy(col[:], ps[:])
        cols.append(col)
    return cols


def build_wa(nc, pool, psum, cp, WT_d, W_d, brow_d, bcol_d, A_ds,
             kin, fout, hw, tag):
    nA = len(A_ds)
    nrhs = fout + hw * nA
    WT = pool.tile([128, kin], f32, tag="bwt")
    nc.sync.dma_start(WT[0:fout, :], WT_d[:])
    WTb = pool.tile([128, kin], bf16, tag="bwtb")
    nc.vector.tensor_copy(WTb[0:fout, :], WT[0:fout, :])
    rhs = cp.tile([128, nrhs], f32, tag="rhs" + tag)
    Wn = pool.tile([128, fout], f32, tag="bwn")
    nc.sync.dma_start(Wn[0:kin, :], W_d[:])
    nc.vector.tensor_copy(rhs[:, 0:fout], Wn[:])
    bx = cp.tile([1, nrhs], f32, tag="bx" + tag)
    bn = pool.tile([1, fout], f32, tag="bbn")
    nc.sync.dma_start(bn[:], brow_d[:])
    nc.vector.tensor_copy(bx[:, 0:fout], bn[:])
    bc = pool.tile([128, 1], f32, tag="bbc")
    nc.sync.dma_start(bc[0:fout, :], bcol_d[:])
    for i, A_d in enumerate(A_ds):
        Ab = pool.tile([128, hw], f32, tag="bab")
        nc.sync.dma_start(Ab[0:fout, :], A_d[:])
        Abb = pool.tile([128, hw], bf16, tag="babb")
        nc.vector.tensor_copy(Abb[0:fout, :], Ab[0:fout, :])
        ps = psum.tile([128, hw], f32, tag="ps")
        nc.tensor.matmul(ps[0:kin, :], WTb[0:fout, 0:kin], Abb[0:fout, :],
                         start=True, stop=True)
        nc.vector.tensor_copy(rhs[:, fout + hw * i:fout + hw * (i + 1)],
                              ps[0:kin, :])
        psb = psum.tile([1, hw], f32, tag="ps")
        nc.tensor.matmul(psb[:], bc[0:fout, 0:1], Ab[0:fout, :], start=True,
                         stop=True)
        nc.vector.tensor_copy(bx[:, fout + hw * i:fout + hw * (i + 1)],
                              psb[:])
    rhsb = cp.tile([128, nrhs], bf16, tag="rhsb" + tag)
    nc.vector.tensor_copy(rhsb[:], rhs[:])
    ones = cp.tile([1, 128], f32, tag="ones" + tag)
    nc.gpsimd.memset(ones[:], 1.0)
    bps = psum.tile([128, nrhs], f32, tag="ps")
    nc.tensor.matmul(bps[:], ones[:], bx[:], start=True, stop=True)
    brep = cp.tile([128, nrhs], f32, tag="brep" + tag)
    nc.vector.tensor_copy(brep[:], bps[:])
    return rhs, rhsb, brep, ones


def emit_asb(nc, pool, psum, cp, ones, fs_d, tag):
    fsb = pool.tile([1, 128], f32, tag="fsb")
    nc.sync.dma_start(fsb[:], fs_d[:])
    ps = psum.tile([128, 128], f32, tag="ps")
    nc.tensor.matmul(ps[:], ones[:], fsb[:], start=True, stop=True)
    asb = cp.tile([128, 128], bf16, tag="asb" + tag)
    nc.vector.tensor_copy(asb[:], ps[:])
    return asb


def emit_proj(nc, pool, psum, spool, xN_ds, rhs_list, brep, nrhs, n_tiles,
              F, S, tbl, dst_tbls=None, iotac=None, dS=0):
    """psum = sum_i xN_i_tile^T @ rhs_i, reading node-major tables via
    transpose-DMA; pack [h|scores] rows into tbl and/or
    [dst-scores f32|dmod f32] rows into dst_tbls."""
    so = 64 if F == 128 else 32
    st = [None]
    dstt = [None]
    for c0 in range(0, n_tiles, 8):
        ntc = min(8, n_tiles - c0)
        xbbs = []
        for xd in xN_ds:
            xbb = pool.tile([128, ntc * 128], bf16, tag="pxb")
            nc.sync.dma_start_transpose(
                xbb[:, 0:ntc * 128], xd[c0 * 128:(c0 + ntc) * 128, :])
            xbbs.append(xbb)
        for t in range(ntc):
            gt = c0 + t
            tl = gt % 16
            if tl == 0:
                if tbl is not None:
                    st[0] = spool.tile([128, 16, 128], bf16, tag="stage",
                                       name="stage")
                    nc.gpsimd.memset(st[0][:], 0.0)
                if dst_tbls:
                    dstt[0] = [spool.tile([128, 16, 128], bf16,
                                          tag=f"dst{i}", name=f"dst{i}")
                               for i in range(len(dst_tbls))]
                    for dd in dstt[0]:
                        nc.gpsimd.memset(dd[:], 0.0)
            ps = psum.tile([128, nrhs], f32, tag="ps")
            for i, xbb in enumerate(xbbs):
                nc.tensor.matmul(ps[:], xbb[:, t * 128:(t + 1) * 128],
                                 rhs_list[i][:], start=(i == 0),
                                 stop=(i == len(xbbs) - 1))
            if tbl is not None:
                nc.vector.tensor_tensor(st[0][:, tl, 0:F], ps[:, 0:F],
                                        brep[:, 0:F], op=ADD)
                if S:
                    nc.vector.tensor_tensor(
                        st[0][:].bitcast(f32)[:, tl, so:so + S],
                        ps[:, F:F + S], brep[:, F:F + S], op=ADD)
            if dst_tbls:
                for i in range(len(dst_tbls)):
                    nc.vector.tensor_tensor(
                        dstt[0][i][:].bitcast(f32)[:, tl, 0:dS],
                        ps[:, F + S + dS * i:F + S + dS * (i + 1)],
                        brep[:, F + S + dS * i:F + S + dS * (i + 1)], op=ADD)
                    nc.vector.tensor_copy(
                        dstt[0][i][:].bitcast(f32)[:, tl, dS:dS + 1],
                        iotac[:])
            if tl == 15 or gt == n_tiles - 1:
                cc = gt - tl
                if tbl is not None:
                    t3 = tbl[0:n_tiles * 128, :].rearrange("(c r) e -> r c e",
                                                           r=128)
                    nc.sync.dma_start(t3[:, cc:cc + tl + 1, :],
                                      st[0][:, 0:tl + 1, :])
                if dst_tbls:
                    for i, db in enumerate(dst_tbls):
                        d3 = db[0:n_tiles * 128, :].rearrange(
                            "(c r) e -> r c e", r=128)
                        nc.sync.dma_start(d3[:, cc:cc + tl + 1, :],
                                          dstt[0][i][:, 0:tl + 1, :])


# ----------------------------------------------------------------- kernels --
def build_k1(meta):
    nc = bacc.Bacc(None, target_bir_lowering=False, debug=False,
                   num_swdge_queues=4)
    dt = nc.dram_tensor
    I, O, N = "ExternalInput", "ExternalOutput", "Internal"
    xaL = dt("xaL", [PAD_A, 128], bf16, kind=I)   # own padded author slice
    xpL = dt("xpL", [PAD_P, 128], bf16, kind=I)   # own padded paper slice
    W1a = dt("W1a", [128, 128], f32, kind=I)
    W1aT = dt("W1aT", [128, 128], f32, kind=I)
    W1p = dt("W1p", [128, 128], f32, kind=I)
    W1pT = dt("W1pT", [128, 128], f32, kind=I)
    b1ar = dt("b1ar", [1, 128], f32, kind=I)
    b1ac = dt("b1ac", [128, 1], f32, kind=I)
    b1pr = dt("b1pr", [1, 128], f32, kind=I)
    b1pc = dt("b1pc", [128, 1], f32, kind=I)
    A = {k: dt("A" + k, [128, 8], f32, kind=I)
         for k in ("dap", "dpa", "daa")}
    FS = {k: dt("fs" + k, [1, 128], f32, kind=I)
          for k in ("ap", "pa", "aa")}
    iota_d = dt("iota", [128, 128], f32, kind=I)
    iotac_d = dt("iotac", [128, 1], f32, kind=I)
    zdrow_d = dt("zdrow", [1, 128], bf16, kind=I)
    Wk1 = dt("Wk1", [128, 128], f32, kind=I)
    bk1 = dt("bk1", [128, 1], f32, kind=I)
    q1 = dt("q1", [128, 1], f32, kind=I)
    mio = {}
    for ty in ("ap", "pa", "aa"):
        nt = meta[ty]["n_tiles"]
        mio[ty] = (dt("s16" + ty, [128, nt * 8], i16, kind=I),
                   dt("d16" + ty, [128, nt * 8], i16, kind=I))
    xaB = dt("xaB", [PAD_A, 128], bf16, kind=N)
    xpB = dt("xpB", [PAD_P, 128], bf16, kind=N)
    xa_f = dt("xa_f", [NA_BLK, 128], bf16, kind=N, addr_space="Shared")
    xp_f = dt("xp_f", [NP_BLK, 128], bf16, kind=N, addr_space="Shared")
    au_t = dt("au_t", [NA_BLK, 128], bf16, kind=N)
    pa_t = dt("pa_t", [NP_BLK, 128], bf16, kind=N)
    apd_t = dt("apd_t", [PAD_P + 128, 128], bf16, kind=N)
    pad_t = dt("pad_t", [PAD_A + 128, 128], bf16, kind=N)
    aad_t = dt("aad_t", [PAD_A + 128, 128], bf16, kind=N)
    o_apL = dt("o_apL", [PAD_P, 128], bf16, kind=O)
    o_paL = dt("o_paL", [PAD_A, 128], bf16, kind=O)
    o_aaL = dt("o_aaL", [PAD_A, 128], bf16, kind=O)
    pw1 = dt("pw1", [1, 2], f32, kind=O)

    with tile.TileContext(nc) as tc:
        with (tc.tile_pool(name="c", bufs=1) as cp,
              tc.tile_pool(name="s", bufs=2) as pool,
              tc.tile_pool(name="st", bufs=2) as spool,
              tc.tile_pool(name="a", bufs=1) as apool,
              tc.tile_pool(name="p", bufs=3, space="PSUM") as psum,
              tc.tile_pool(name="p2", bufs=5, space="PSUM") as psum2):
            nc.sync.dma_start(xaB[:], xaL[:])
            nc.sync.dma_start(xpB[:], xpL[:])
            nc.gpsimd.collective_compute("AllGather", BYP, GRP,
                                         ins=[xaB[:]], outs=[xa_f[:]])
            nc.gpsimd.collective_compute("AllGather", BYP, GRP,
                                         ins=[xpB[:]], outs=[xp_f[:]])
            iot = cp.tile([128, 128], f32)
            nc.sync.dma_start(iot[:], iota_d[:])
            iotac = cp.tile([128, 1], f32)
            nc.sync.dma_start(iotac[:], iotac_d[:])
            idf = cp.tile([128, 128], f32)
            nc.vector.tensor_scalar(out=idf[:], in0=iot[:],
                                    scalar1=iotac[:, 0:1], scalar2=None,
                                    op0=EQ)
            zdrow = cp.tile([1, 128], bf16)
            nc.sync.dma_start(zdrow[:], zdrow_d[:])
            for tb, wn in ((apd_t, W_P), (pad_t, W_A), (aad_t, W_A)):
                nc.sync.dma_start(tb[wn * 128:wn * 128 + 1, :], zdrow[:])

            _, ra, bra, ones = build_wa(nc, pool, psum, cp, W1aT, W1a, b1ar,
                                        b1ac, [], 128, 128, 8, "a")
            _, rp, brp, _ = build_wa(nc, pool, psum, cp, W1pT, W1p, b1pr,
                                     b1pc, [], 128, 128, 8, "p")
            _, rpd, brpd, _ = build_wa(nc, pool, psum, cp, W1pT, W1p, b1pr,
                                       b1pc, [A["dap"]], 128, 128, 8, "pd")
            _, rad, brad, _ = build_wa(nc, pool, psum, cp, W1aT, W1a, b1ar,
                                       b1ac, [A["dpa"], A["daa"]],
                                       128, 128, 8, "ad")
            asb_ap = emit_asb(nc, pool, psum, cp, ones, FS["ap"], "ap")
            asb_pa = emit_asb(nc, pool, psum, cp, ones, FS["pa"], "pa")
            asb_aa = emit_asb(nc, pool, psum, cp, ones, FS["aa"], "aa")

            emit_proj(nc, pool, psum, spool, [xa_f], [ra], bra, 128,
                      NA_BLK // 128, 128, 0, au_t)
            emit_proj(nc, pool, psum, spool, [xp_f], [rp], brp, 128,
                      NP_BLK // 128, 128, 0, pa_t)
            emit_proj(nc, pool, psum, spool, [xpL], [rpd], brpd, 136, W_P,
                      128, 0, None, dst_tbls=[apd_t], iotac=iotac, dS=8)
            emit_proj(nc, pool, psum, spool, [xaL], [rad], brad, 144, W_A,
                      128, 0, None, dst_tbls=[pad_t, aad_t],
                      iotac=iotac, dS=8)

            acc = apool.tile([128, W_P * 136], f32, tag="acc")
            nc.gpsimd.memset(acc[:], 0.0)
            emit_edge_phase(nc, pool, psum2, au_t, apd_t, *mio["ap"],
                            meta["ap"], 128, 8, acc, [0, CHK], iot,
                            asb=asb_ap)
            emit_normalize(nc, pool, acc, W_P, 128, 8, o_apL[:], bf16)
            acc = apool.tile([128, W_A * 136], f32, tag="acc")
            nc.gpsimd.memset(acc[:], 0.0)
            emit_edge_phase(nc, pool, psum2, pa_t, pad_t, *mio["pa"],
                            meta["pa"], 128, 8, acc,
                            [0, CHK, 2 * CHK, 3 * CHK], iot, asb=asb_pa)
            emit_normalize(nc, pool, acc, W_A, 128, 8, o_paL[:], bf16)
            acc = apool.tile([128, W_A * 136], f32, tag="acc")
            nc.gpsimd.memset(acc[:], 0.0)
            emit_edge_phase(nc, pool, psum2, au_t, aad_t, *mio["aa"],
                            meta["aa"], 128, 8, acc, [0, CHK], iot,
                            asb=asb_aa)
            emit_normalize(nc, pool, acc, W_A, 128, 8, o_aaL[:], bf16)

            pw = emit_tanh_partial(nc, pool, psum2,
                                   [(o_paL, PAD_A, False),
                                    (o_aaL, PAD_A, False)],
                                   Wk1, bk1, q1, 128, idf, PAD_A - SL_A)
            nc.sync.dma_start(pw1[:], pw[:])
    nc.compile()
    return nc


def build_k2(meta):
    nc = bacc.Bacc(None, target_bir_lowering=False, debug=False,
                   num_swdge_queues=4)
    dt = nc.dram_tensor
    I, O, N = "ExternalInput", "ExternalOutput", "Internal"
    o_apL = dt("o_apL", [PAD_P, 128], bf16, kind=I)
    o_paL = dt("o_paL", [PAD_A, 128], bf16, kind=I)
    o_aaL = dt("o_aaL", [PAD_A, 128], bf16, kind=I)
    pw1 = dt("pw1", [1, 2], f32, kind=I)
    W2a = dt("W2a", [128, 64], f32, kind=I)
    W2aT = dt("W2aT", [64, 128], f32, kind=I)
    W2p = dt("W2p", [128, 64], f32, kind=I)
    W2pT = dt("W2pT", [64, 128], f32, kind=I)
    b2ar = dt("b2ar", [1, 64], f32, kind=I)
    b2ac = dt("b2ac", [64, 1], f32, kind=I)
    b2pr = dt("b2pr", [1, 64], f32, kind=I)
    b2pc = dt("b2pc", [64, 1], f32, kind=I)
    A2 = {k: dt("A2" + k, [64, 4], f32, kind=I)
          for k in ("spa", "dpa", "saa", "daa")}
    iota_d = dt("iota", [128, 128], f32, kind=I)
    iotac_d = dt("iotac", [128, 1], f32, kind=I)
    zdrow_d = dt("zdrow", [1, 128], bf16, kind=I)
    Wk2 = dt("Wk2", [64, 64], f32, kind=I)
    bk2 = dt("bk2", [64, 1], f32, kind=I)
    q2 = dt("q2", [64, 1], f32, kind=I)
    mio = {}
    for ty in ("pa", "aa"):
        nt = meta[ty]["n_tiles"]
        mio[ty] = (dt("s16" + ty, [128, nt * 8], i16, kind=I),
                   dt("d16" + ty, [128, nt * 8], i16, kind=I))
    oapB = dt("oapB", [PAD_P, 128], bf16, kind=N)
    opaB = dt("opaB", [PAD_A, 128], bf16, kind=N)
    oaaB = dt("oaaB", [PAD_A, 128], bf16, kind=N)
    pw1B = dt("pw1B", [1, 2], f32, kind=N)
    oap_f = dt("oap_f", [NP_BLK, 128], bf16, kind=N, addr_space="Shared")
    opa_f = dt("opa_f", [NA_BLK, 128], bf16, kind=N, addr_space="Shared")
    oaa_f = dt("oaa_f", [NA_BLK, 128], bf16, kind=N, addr_space="Shared")
    pw1g = dt("pw1g", [NC, 2], f32, kind=N, addr_space="Shared")
    pw2b = dt("pw2b", [1, 2], f32, kind=N)
    pw2s = dt("pw2s", [1, 2], f32, kind=N, addr_space="Shared")
    au_t = dt("au_t", [NA_BLK, 128], bf16, kind=N)
    pa_t = dt("pa_t", [NP_BLK, 128], bf16, kind=N)
    pad_t = dt("pad_t", [PAD_A + 128, 128], bf16, kind=N)
    aad_t = dt("aad_t", [PAD_A + 128, 128], bf16, kind=N)
    o2pa = dt("o2pa", [PAD_A, 64], f32, kind=N)
    o2aa = dt("o2aa", [PAD_A, 64], f32, kind=N)
    out = dt("out", [PAD_A, 64], f32, kind=O)

    with tile.TileContext(nc) as tc:
        with (tc.tile_pool(name="c", bufs=1) as cp,
              tc.tile_pool(name="s", bufs=2) as pool,
              tc.tile_pool(name="st", bufs=2) as spool,
              tc.tile_pool(name="a", bufs=1) as apool,
              tc.tile_pool(name="p", bufs=3, space="PSUM") as psum,
              tc.tile_pool(name="p2", bufs=5, space="PSUM") as psum2):
            nc.sync.dma_start(oapB[:], o_apL[:])
            nc.sync.dma_start(opaB[:], o_paL[:])
            nc.sync.dma_start(oaaB[:], o_aaL[:])
            nc.sync.dma_start(pw1B[:], pw1[:])
            nc.gpsimd.collective_compute("AllGather", BYP, GRP,
                                         ins=[oapB[:]], outs=[oap_f[:]])
            nc.gpsimd.collective_compute("AllGather", BYP, GRP,
                                         ins=[opaB[:]], outs=[opa_f[:]])
            nc.gpsimd.collective_compute("AllGather", BYP, GRP,
                                         ins=[oaaB[:]], outs=[oaa_f[:]])
            nc.gpsimd.collective_compute("AllGather", BYP, GRP,
                                         ins=[pw1B[:]], outs=[pw1g[:]])
            iot = cp.tile([128, 128], f32)
            nc.sync.dma_start(iot[:], iota_d[:])
            iotac = cp.tile([128, 1], f32)
            nc.sync.dma_start(iotac[:], iotac_d[:])
            idf = cp.tile([128, 128], f32)
            nc.vector.tensor_scalar(out=idf[:], in0=iot[:],
                                    scalar1=iotac[:, 0:1], scalar2=None,
                                    op0=EQ)
            zdrow = cp.tile([1, 128], bf16)
            nc.sync.dma_start(zdrow[:], zdrow_d[:])
            for tb in (pad_t, aad_t):
                nc.sync.dma_start(tb[W_A * 128:W_A * 128 + 1, :], zdrow[:])

            ra_f, _, bra, ones = build_wa(nc, pool, psum, cp, W2aT, W2a,
                                          b2ar, b2ac, [A2["saa"]],
                                          128, 64, 4, "a")
            _, rp, brp, _ = build_wa(nc, pool, psum, cp, W2pT, W2p, b2pr,
                                     b2pc, [A2["spa"]], 128, 64, 4, "p")
            rad_f, _, brad, _ = build_wa(nc, pool, psum, cp, W2aT, W2a, b2ar,
                                         b2ac, [A2["dpa"], A2["daa"]],
                                         128, 64, 4, "ad")
            # beta1 from gathered pw1: s = sum_c pw1g[c, :] / N_A
            pwt = pool.tile([8, 2], f32, tag="pwt")
            nc.sync.dma_start(pwt[:], pw1g[:])
            ones8 = cp.tile([8, 1], f32, tag="ones8")
            nc.gpsimd.memset(ones8[:], 1.0)
            ps_s = psum.tile([1, 2], f32, tag="ps")
            nc.tensor.matmul(ps_s[:], ones8[:], pwt[:], start=True,
                             stop=True)
            s1 = pool.tile([1, 2], f32, tag="s1")
            nc.vector.tensor_scalar(out=s1[:], in0=ps_s[:],
                                    scalar1=1.0 / N_A, scalar2=None,
                                    op0=MULT)
            bcols = emit_beta_tail(nc, pool, psum, s1, ones)
            ra_s, rad_s = [], []
            for m in range(2):
                rs = cp.tile([128, 68], bf16, tag=f"ras{m}")
                nc.scalar.activation(rs[:], ra_f[:], AF.Copy,
                                     scale=bcols[m][:])
                ra_s.append(rs)
                rds = cp.tile([128, 72], bf16, tag=f"rads{m}")
                nc.scalar.activation(rds[:], rad_f[:], AF.Copy,
                                     scale=bcols[m][:])
                rad_s.append(rds)

            emit_proj(nc, pool, psum, spool, [opa_f, oaa_f], ra_s, bra, 68,
                      NA_BLK // 128, 64, 4, au_t)
            emit_proj(nc, pool, psum, spool, [oap_f], [rp], brp, 68,
                      NP_BLK // 128, 64, 4, pa_t)
            emit_proj(nc, pool, psum, spool, [o_paL, o_aaL], rad_s, brad, 72,
                      W_A, 64, 0, None, dst_tbls=[pad_t, aad_t],
                      iotac=iotac, dS=4)

            acc = apool.tile([128, W_A * 68], f32, tag="acc")
            nc.gpsimd.memset(acc[:], 0.0)
            emit_edge_phase(nc, pool, psum2, pa_t, pad_t, *mio["pa"],
                            meta["pa"], 64, 4, acc,
                            [0, CHK, 2 * CHK, 3 * CHK], iot, so=32)
            emit_normalize(nc, pool, acc, W_A, 64, 4, o2pa[:], f32)
            acc = apool.tile([128, W_A * 68], f32, tag="acc")
            nc.gpsimd.memset(acc[:], 0.0)
            emit_edge_phase(nc, pool, psum2, au_t, aad_t, *mio["aa"],
                            meta["aa"], 64, 4, acc, [0, CHK], iot, so=32)
            emit_normalize(nc, pool, acc, W_A, 64, 4, o2aa[:], f32)

            pw = emit_tanh_partial(nc, pool, psum2,
                                   [(o2pa, PAD_A, True), (o2aa, PAD_A, True)],
                                   Wk2, bk2, q2, 64, idf, PAD_A - SL_A)
            nc.gpsimd.dma_start(pw2b[:], pw[:])
            nc.gpsimd.collective_compute("AllReduce", ADD, GRP,
                                         ins=[pw2b[:]], outs=[pw2s[:]])
            pw2t = pool.tile([1, 2], f32, tag="pw2t")
            nc.sync.dma_start(pw2t[:], pw2s[:])
            s2 = pool.tile([1, 2], f32, tag="s2")
            nc.vector.tensor_scalar(out=s2[:], in0=pw2t[:],
                                    scalar1=1.0 / N_A, scalar2=None,
                                    op0=MULT)
            bcols2 = emit_beta_tail(nc, pool, psum, s2, ones)
            for t in range(PAD_A // 128):
                a = pool.tile([128, 64], f32, tag="ta")
                b = pool.tile([128, 64], f32, tag="tb")
                nc.sync.dma_start(a[:], o2pa[t * 128:(t + 1) * 128, :])
                nc.sync.dma_start(b[:], o2aa[t * 128:(t + 1) * 128, :])
                nc.vector.tensor_scalar(out=a[:], in0=a[:],
                                        scalar1=bcols2[0][:, 0:1],
                                        scalar2=None, op0=MULT)
                nc.vector.tensor_scalar(out=b[:], in0=b[:],
                                        scalar1=bcols2[1][:, 0:1],
                                        scalar2=None, op0=MULT)
                nc.vector.tensor_tensor(a[:], a[:], b[:], op=ADD)
                nc.sync.dma_start(out[t * 128:(t + 1) * 128, :], a[:])
    nc.compile()
    return nc


# ------------------------------------------------------------------ driver --
DBG = {}
EXEC_NS = []


def _make_runner(nc, mesh, sh):
    """jit-of-shard_map wrapper for one Bass program (axon bass_exec path)."""
    import jax
    from jax.sharding import PartitionSpec
    from concourse.bass2jax import (_bass_exec_p, partition_id_tensor,
                                    install_neuronx_cc_hook)
    install_neuronx_cc_hook()
    pname = nc.partition_id_tensor.name if nc.partition_id_tensor else None
    in_names, out_names, out_avals, zero_shapes = [], [], [], []
    for alloc in nc.m.functions[0].allocations:
        if not isinstance(alloc, mybir.MemoryLocationSet):
            continue
        name = alloc.memorylocations[0].name
        if alloc.kind == "ExternalInput":
            if name != pname:
                in_names.append(name)
        elif alloc.kind == "ExternalOutput":
            out_names.append(name)
            shape = tuple(alloc.tensor_shape)
            dtype = mybir.dt.np(alloc.dtype)
            out_avals.append(jax.core.ShapedArray(shape, dtype))
            zero_shapes.append((shape, dtype))
    n_params, n_outs = len(in_names), len(out_avals)
    all_in = list(in_names) + list(out_names)
    if pname is not None:
        all_in.append(pname)

    def _body(*args):
        operands = list(args)
        if pname is not None:
            operands.append(partition_id_tensor())
        return tuple(_bass_exec_p.bind(
            *operands, out_avals=tuple(out_avals), in_names=tuple(all_in),
            out_names=tuple(out_names), lowering_input_output_aliases=(),
            sim_require_finite=True, sim_require_nnan=True, nc=nc))

    spec = PartitionSpec("core")
    # no donation: every ExternalOutput is fully written, so the zero
    # "output seed" operands are dead inputs reusable across repetitions.
    fn = jax.jit(
        jax.shard_map(_body, mesh=mesh, in_specs=(spec,) * (n_params + n_outs),
                      out_specs=(spec,) * n_outs, check_vma=False),
        keep_unused=True)
    return fn, in_names, out_names, zero_shapes


def kernel(**inp):
    import time
    import jax
    from jax.sharding import Mesh, PartitionSpec, NamedSharding

    inp = {k: np.asarray(v) for k, v in inp.items()}
    m1 = {"ap": prep_type(inp["ei_ap_src"], inp["ei_ap_dst"], N_P,
                          SL_A, PAD_A, W_P),
          "pa": prep_type(inp["ei_pa_src"], inp["ei_pa_dst"], N_A,
                          SL_P, PAD_P, W_A),
          "aa": prep_type(inp["ei_aa_src"], inp["ei_aa_dst"], N_A,
                          SL_A, PAD_A, W_A)}
    iota = np.tile(np.arange(128, dtype=np.float32), (128, 1))
    iotac = np.arange(128, dtype=np.float32)[:, None]
    zdrow_f = np.zeros(64, np.float32)
    zdrow_f[4] = -1.0   # layer-2 dmod slot
    zdrow_f[8] = -1.0   # layer-1 dmod slot
    zdrow = zdrow_f.view(BF)[None, :]

    devices = jax.devices()[:NC]
    mesh = Mesh(np.asarray(devices), ("core",))
    P = PartitionSpec
    sh = NamedSharding(mesh, P("core"))

    nc1 = build_k1(m1)
    nc2 = build_k2({"pa": m1["pa"], "aa": m1["aa"]})
    f1, in1_names, out1_names, zs1 = _make_runner(nc1, mesh, sh)
    f2, in2_names, out2_names, zs2 = _make_runner(nc2, mesh, sh)

    def put_repl(x):
        x = np.asarray(x)
        shards = [jax.device_put(x, d) for d in devices]
        gshape = (NC * x.shape[0],) + x.shape[1:]
        return jax.make_array_from_single_device_arrays(gshape, sh, shards)

    def put_percore(xs):
        xs = [np.asarray(x) for x in xs]
        shards = [jax.device_put(x, d) for x, d in zip(xs, devices)]
        gshape = (NC * xs[0].shape[0],) + xs[0].shape[1:]
        return jax.make_array_from_single_device_arrays(gshape, sh, shards)

    const = dict(
        W1a=inp["W1_a"], W1aT=np.ascontiguousarray(inp["W1_a"].T),
        W1p=inp["W1_p"], W1pT=np.ascontiguousarray(inp["W1_p"].T),
        b1ar=inp["b1_a"][None, :], b1ac=inp["b1_a"][:, None],
        b1pr=inp["b1_p"][None, :], b1pc=inp["b1_p"][:, None],
        Adap=ablk(inp["a1d_ap"], 128), Adpa=ablk(inp["a1d_pa"], 128),
        Adaa=ablk(inp["a1d_aa"], 128),
        fsap=inp["a1s_ap"].reshape(1, 128),
        fspa=inp["a1s_pa"].reshape(1, 128),
        fsaa=inp["a1s_aa"].reshape(1, 128),
        iota=iota, iotac=iotac, zdrow=zdrow,
        Wk1=inp["Wk1"], bk1=inp["bk1"][:, None], q1=inp["q1"][:, None],
        W2a=inp["W2_a"], W2aT=np.ascontiguousarray(inp["W2_a"].T),
        W2p=inp["W2_p"], W2pT=np.ascontiguousarray(inp["W2_p"].T),
        b2ar=inp["b2_a"][None, :], b2ac=inp["b2_a"][:, None],
        b2pr=inp["b2_p"][None, :], b2pc=inp["b2_p"][:, None],
        A2spa=ablk(inp["a2s_pa"], 64), A2dpa=ablk(inp["a2d_pa"], 64),
        A2saa=ablk(inp["a2s_aa"], 64), A2daa=ablk(inp["a2d_aa"], 64),
        Wk2=inp["Wk2"], bk2=inp["bk2"][:, None], q2=inp["q2"][:, None])
    dev = {k: put_repl(v) for k, v in const.items()}
    for ty in ("ap", "pa", "aa"):
        dev["s16" + ty] = put_percore(m1[ty]["s16"])
        dev["d16" + ty] = put_percore(m1[ty]["d16"])
    xa = inp["x_author"].astype(BF)
    xp = inp["x_paper"].astype(BF)
    dev["xaL"] = put_percore([
        np.pad(xa[c * SL_A:(c + 1) * SL_A], ((0, PAD_A - SL_A), (0, 0)))
        for c in range(NC)])
    dev["xpL"] = put_percore([
        np.pad(xp[c * SL_P:(c + 1) * SL_P], ((0, PAD_P - SL_P), (0, 0)))
        for c in range(NC)])
    jax.block_until_ready(list(dev.values()))

    def zeros(zshapes):
        z = [put_percore([np.zeros(s, d)] * NC) for s, d in zshapes]
        jax.block_until_ready(z)
        return z

    z1, z2 = zeros(zs1), zeros(zs2)

    def chain():
        o1 = f1(*[dev[n] for n in in1_names], *z1)
        o1m = dict(zip(out1_names, o1))
        io2 = dict(dev, **o1m)
        o2 = f2(*[io2[n] for n in in2_names], *z2)
        return o2[out2_names.index("out")]

    out_d = chain()
    jax.block_until_ready(out_d)

    # timed: N complete model executions enqueued back-to-back, one sync
    N = 64
    t0 = time.time()
    outs = [chain() for _ in range(N)]
    out_d = outs[-1]
    jax.block_until_ready(outs)
    EXEC_NS.append(int((time.time() - t0) * 1e9) // N)

    out_g = np.asarray(out_d)
    out = np.zeros((N_A, 64), np.float32)
    for c in range(NC):
        out[c * SL_A:(c + 1) * SL_A] = out_g[c * PAD_A:c * PAD_A + SL_A]
    return out


# revision 15
# speedup vs baseline: 1.0383x; 1.0383x over previous
"""HAN (2-layer heterogeneous GAT) on 8 Trainium2 NeuronCores (Bass/Tile).

v4: two launches total.  Node tables live in block layout (node id ->
blk*SLB + local); every table row is 256B.  k1: AllGather x slices ->
project (reading node-major tables through transpose-DMA) -> edge softmax
(src rows carry h only, es computed on-chip; dst rows carry
[scores f32|dmod f32]; one-hot scatter built on-chip via is_equal(iota,
dmod); gathers spread over 4 SWDGE queues) -> per-slice outputs + pw1
partial.  k2: AllGather layer-1 outputs + pw1, semantic-attention beta
on-chip, layer 2 (same structure), AllReduce pw2, final beta combine.

Compile + staging are untimed; EXEC_NS reports the mean per-execution wall
time of N pipelined repetitions of the k1->k2 chain.
"""
import numpy as np
import ml_dtypes

import concourse.bacc as bacc
import concourse.tile as tile
import concourse.mybir as mybir
from concourse import bass_utils  # noqa: F401

BF = ml_dtypes.bfloat16
N_A, N_P, E, NC = 50000, 100000, 800000, 8
SL_A, SL_P = N_A // NC, N_P // NC                # 6250, 12500
W_A, W_P = (SL_A + 127) // 128, (SL_P + 127) // 128  # 49, 98
PAD_A, PAD_P = W_A * 128, W_P * 128              # 6272, 12544 (= SLB)
NA_BLK, NP_BLK = NC * PAD_A, NC * PAD_P          # 50176, 100352
CHK = 32768
EPS = 1e-6
CT = 40                                          # tiles per device chunk

f32, bf16, i16 = mybir.dt.float32, mybir.dt.bfloat16, mybir.dt.int16
ADD, MULT, MAX = mybir.AluOpType.add, mybir.AluOpType.mult, mybir.AluOpType.max
EQ = mybir.AluOpType.is_equal
BYP = mybir.AluOpType.bypass
AF = mybir.ActivationFunctionType
GRP = [list(range(NC))]


# ---------------------------------------------------------------- host prep --
def pack16(idx):
    t = np.ascontiguousarray(idx.reshape(-1, 16).T.astype(np.int16))
    return np.tile(t, (8, 1))


def prep_type(src, dst, n_dst, src_sl, src_slb, n_win):
    """Uniform-schedule edge prep for one edge type across all 8 cores.

    src ids are remapped into block layout: id -> (id//src_sl)*src_slb +
    id%src_sl.  dst ids stay window-local within the owning core's slice."""
    sl = n_dst // NC
    n_src_rows = NC * src_slb
    n_chk = (n_src_rows + CHK - 1) // CHK
    K = n_chk * n_win
    sent = n_win * 128
    per = []
    for c in range(NC):
        m = (dst >= c * sl) & (dst < (c + 1) * sl)
        es = src[m].astype(np.int64)
        es = (es // src_sl) * src_slb + es % src_sl
        ed = (dst[m] - c * sl).astype(np.int64)
        key = (es // CHK) * n_win + (ed >> 7)
        o = np.argsort(key, kind="stable")
        per.append((es[o], ed[o], key[o]))
    cnts = np.stack([np.bincount(p[2], minlength=K) for p in per])
    T = (cnts.max(0) + 127) // 128
    keys = np.nonzero(T)[0]
    offs = np.zeros(K + 1, np.int64)
    offs[1:] = np.cumsum(T) * 128
    n_tiles = int(T.sum())
    npad = n_tiles * 128
    tw, tfirst, tlast, tcopy = [], [], [], []
    seen = set()
    for k in keys:
        w = int(k % n_win)
        nt = int(T[k])
        tw += [w] * nt
        tfirst += [True] + [False] * (nt - 1)
        tlast += [False] * (nt - 1) + [True]
        tcopy += [w not in seen] * nt
        seen.add(w)
    tchk = np.repeat(keys // n_win, T[keys])
    segs = []
    for c0 in range(0, n_tiles, CT):
        nt = min(CT, n_tiles - c0)
        cs, t = [], 0
        while t < nt:
            cb = int(tchk[c0 + t])
            t2 = t
            while t2 < nt and tchk[c0 + t2] == cb:
                t2 += 1
            cs.append((t, t2 - t, cb))
            t = t2
        segs.append(cs)
    s16, d16 = [], []
    for es, ed, key in per:
        sa = np.zeros(npad, np.int64)
        da = np.full(npad, sent, np.int64)
        st, cn = np.unique(key, return_index=True)
        cnt = np.diff(np.append(cn, len(key)))
        for k, s0, c_ in zip(st, cn, cnt):
            off = offs[k]
            sa[off:off + c_] = es[s0:s0 + c_] - (k // n_win) * CHK
            da[off:off + c_] = ed[s0:s0 + c_]
        s16.append(pack16(sa))
        d16.append(pack16(da))
    return dict(n_tiles=n_tiles, tw=tw, tfirst=tfirst, tlast=tlast,
                tcopy=tcopy, segs=segs, s16=s16, d16=d16)


def ablk(a, F):
    H = a.shape[0]
    o = np.zeros((F, H), np.float32)
    for h in range(H):
        o[h * 16:(h + 1) * 16, h] = a[h]
    return o


# ------------------------------------------------------------ device pieces --
def emit_edge_phase(nc, pool, psum, src_tbl, dst_tbl, s16d, d16d, meta,
                    F, H, accum, chunk_bases, iot, asb=None, so=0):
    """Edge softmax-accumulate for one edge type (see module docstring)."""
    NR = F + H
    n_tiles = meta["n_tiles"]
    tw, tf, tl, tc = meta["tw"], meta["tfirst"], meta["tlast"], meta["tcopy"]
    cur = [None]
    nrows = src_tbl.shape[0]
    for ci, c0 in enumerate(range(0, n_tiles, CT)):
        nt = min(CT, n_tiles - c0)
        si = pool.tile([128, nt * 8], i16, tag="si")
        di = pool.tile([128, nt * 8], i16, tag="di")
        nc.sync.dma_start(si[:], s16d[:, c0 * 8:(c0 + nt) * 8])
        nc.sync.dma_start(di[:], d16d[:, c0 * 8:(c0 + nt) * 8])
        G = pool.tile([128, nt, 128], bf16, tag="G")
        D = pool.tile([128, nt, 128], bf16, tag="D")
        for (t0, tn, cb) in meta["segs"][ci]:
            b = chunk_bases[cb]
            nc.gpsimd.dma_gather(
                out_ap=G[:, t0:t0 + tn, :],
                in_ap=src_tbl[b:min(b + CHK, nrows), :],
                idxs_ap=si[:, t0 * 8:(t0 + tn) * 8],
                num_idxs=tn * 128, num_idxs_reg=tn * 128, elem_size=128,
                single_packet=False, queue_num=ci % 2)
        nc.gpsimd.dma_gather(
            out_ap=D[:, 0:nt, :], in_ap=dst_tbl[:], idxs_ap=di[:],
            num_idxs=nt * 128, num_idxs_reg=nt * 128, elem_size=128,
            single_packet=False, queue_num=2 + ci % 2)
        Gf, Df = G[:].bitcast(f32), D[:].bitcast(f32)
        es = pool.tile([128, nt, H], f32, tag="es")
        scr = pool.tile([128, nt, 128], bf16, tag="scr")
        if asb is not None:
            nc.vector.tensor_tensor(
                scr[:], G[:, 0:nt, :],
                asb[:, None, :].broadcast_to([128, nt, 128]), op=MULT)
            nc.vector.tensor_reduce(
                es[:], scr[:].rearrange("p t (h d) -> p t h d", h=H),
                axis=mybir.AxisListType.X, op=ADD)
        else:
            nc.vector.tensor_copy(es[:], Gf[:, 0:nt, so:so + H])
        al = pool.tile([128, nt, H], f32, tag="al")
        nc.vector.tensor_tensor(al[:], es[:], Df[:, 0:nt, 0:H], op=ADD)
        lr = pool.tile([128, nt, H], f32, tag="lr")
        nc.vector.tensor_scalar(out=lr[:], in0=al[:], scalar1=0.2,
                                scalar2=None, op0=MULT)
        nc.vector.tensor_tensor(lr[:], lr[:], al[:], op=MAX)
        w = pool.tile([128, nt, H], f32, tag="w")
        nc.scalar.activation(w[:], lr[:], AF.Exp)
        M3 = scr
        nc.vector.tensor_tensor(
            M3[:], iot[:, None, :].broadcast_to([128, nt, 128]),
            Df[:, 0:nt, H:H + 1].broadcast_to([128, nt, 128]), op=EQ)
        VW = pool.tile([128, nt, NR], bf16, tag="VW")
        nc.vector.tensor_tensor(
            VW[:, :, 0:F].rearrange("p t (h d) -> p t h d", h=H),
            G[:, 0:nt, 0:F].rearrange("p t (h d) -> p t h d", h=H),
            w[:, :, :, None].broadcast_to([128, nt, H, 16]), op=MULT)
        nc.vector.tensor_copy(VW[:, :, F:NR], w[:])
        for t in range(nt):
            g = c0 + t
            if tf[g]:
                cur[0] = psum.tile([128, NR], f32, tag="eps", name="eps")
            nc.tensor.matmul(cur[0][:], M3[:, t, :], VW[:, t, :],
                             start=tf[g], stop=tl[g])
            if tl[g]:
                ws = accum[:, tw[g] * NR:(tw[g] + 1) * NR]
                if tc[g]:
                    nc.vector.tensor_copy(ws, cur[0][:])
                else:
                    nc.vector.tensor_tensor(ws, ws, cur[0][:], op=ADD)


def emit_normalize(nc, pool, accum, n_win, F, H, o_out, odt):
    NR = F + H
    a3 = accum.rearrange("p (w r) -> p w r", r=NR)
    o3 = o_out.rearrange("(w r) f -> r w f", r=128)
    for w0 in range(0, n_win, 4):
        nw = min(4, n_win - w0)
        rc = pool.tile([128, nw, H], f32, tag="rc")
        nc.vector.tensor_scalar(out=rc[:], in0=a3[:, w0:w0 + nw, F:NR],
                                scalar1=EPS, scalar2=None, op0=ADD)
        nc.vector.reciprocal(rc[:], rc[:])
        ot = pool.tile([128, nw, F], odt, tag="ot")
        nc.vector.tensor_tensor(
            ot[:].rearrange("p w (h d) -> p w h d", h=H),
            a3[:, w0:w0 + nw, 0:F].rearrange("p w (h d) -> p w h d", h=H),
            rc[:, :, :, None].broadcast_to([128, nw, H, 16]), op=MULT)
        nc.vector.tensor_scalar(out=ot[:], in0=ot[:], scalar1=0.0,
                                scalar2=None, op0=MAX)
        nc.sync.dma_start(o3[:, w0:w0 + nw, :], ot[:])


def emit_tanh_partial(nc, pool, psum, o_list, Wk_d, bk_d, q_d, F, ident,
                      n_pad_rows):
    Wkf = pool.tile([128, F], f32, tag="wkf")
    nc.sync.dma_start(Wkf[0:F, :], Wk_d[:])
    Wk = pool.tile([128, F], bf16, tag="wkb")
    nc.vector.tensor_copy(Wk[0:F, :], Wkf[0:F, :])
    bk = pool.tile([128, 1], f32, tag="bk")
    nc.sync.dma_start(bk[0:F, :], bk_d[:])
    qf = pool.tile([128, 1], f32, tag="qf")
    nc.sync.dma_start(qf[0:F, :], q_d[:])
    q = pool.tile([128, 1], bf16, tag="qb")
    nc.vector.tensor_copy(q[0:F, :], qf[0:F, :])
    tb = pool.tile([128, 1], f32, tag="tbk")
    nc.scalar.activation(tb[0:F, :], bk[0:F, :], AF.Tanh)
    corr_ps = psum.tile([1, 1], f32, tag="eps")
    nc.tensor.matmul(corr_ps[:], qf[0:F, 0:1], tb[0:F, :], start=True,
                     stop=True)
    corr = pool.tile([1, 1], f32, tag="corr")
    nc.vector.tensor_scalar(out=corr[:], in0=corr_ps[:],
                            scalar1=-float(n_pad_rows), scalar2=None,
                            op0=MULT)
    pw = pool.tile([1, 2], f32, tag="pw")
    for m, (o_d, npad, isf32) in enumerate(o_list):
        qacc = pool.tile([1, 128], f32, tag="qacc")
        nc.gpsimd.memset(qacc[:], 0.0)
        for t in range(npad // 128):
            oT = pool.tile([128, 128], bf16, tag="oT")
            if not isf32:
                nc.sync.dma_start_transpose(oT[0:F, :],
                                            o_d[t * 128:(t + 1) * 128, :])
            else:
                of = pool.tile([128, F], f32, tag="of")
                nc.sync.dma_start(of[:], o_d[t * 128:(t + 1) * 128, :])
                tp = psum.tile([128, 128], f32, tag="eps")
                nc.tensor.transpose(tp[0:F, :], of[:], ident[:])
                nc.vector.tensor_copy(oT[0:F, :], tp[0:F, :])
            ps2 = psum.tile([128, 128], f32, tag="eps")
            nc.tensor.matmul(ps2[0:F, :], Wk[0:F, :], oT[0:F, :],
                             start=True, stop=True)
            th = pool.tile([128, 128], bf16, tag="th")
            nc.scalar.activation(th[0:F, :], ps2[0:F, :], AF.Tanh,
                                 bias=bk[0:F, :])
            ps3 = psum.tile([1, 128], f32, tag="eps")
            nc.tensor.matmul(ps3[:], q[0:F, 0:1], th[0:F, :], start=True,
                             stop=True)
            nc.vector.tensor_tensor(qacc[:], qacc[:], ps3[:], op=ADD)
        red = pool.tile([1, 1], f32, tag="red")
        nc.vector.tensor_reduce(red[:], qacc[:], axis=mybir.AxisListType.X,
                                op=ADD)
        nc.vector.tensor_tensor(pw[0:1, m:m + 1], red[:], corr[:], op=ADD)
    return pw


def emit_beta_tail(nc, pool, psum, s, ones):
    """s: [1,2] SBUF tile of summed per-metapath scores (already /N)."""
    e = pool.tile([1, 2], f32, tag="pt3")
    nc.scalar.activation(e[:], s[:], AF.Exp)
    dn = pool.tile([1, 1], f32, tag="pt4")
    nc.vector.tensor_reduce(dn[:], e[:], axis=mybir.AxisListType.X, op=ADD)
    rcp = pool.tile([1, 1], f32, tag="pt5")
    nc.vector.reciprocal(rcp[:], dn[:])
    beta = pool.tile([1, 2], f32, tag="pt6")
    nc.vector.tensor_tensor(beta[:], e[:], rcp[:].broadcast_to([1, 2]),
                            op=MULT)
    cols = []
    for m in range(2):
        ps = psum.tile([128, 1], f32, tag="ps")
        nc.tensor.matmul(ps[:], ones[:], beta[0:1, m:m + 1], start=True,
                         stop=True)
        col = pool.tile([128, 1], f32, tag=f"bcol{m}")
        nc.vector.tensor_copy(col[:], ps[:])
        cols.append(col)
    return cols


def build_wa(nc, pool, psum, cp, WT_d, W_d, brow_d, bcol_d, A_ds,
             kin, fout, hw, tag):
    nA = len(A_ds)
    nrhs = fout + hw * nA
    WT = pool.tile([128, kin], f32, tag="bwt")
    nc.sync.dma_start(WT[0:fout, :], WT_d[:])
    WTb = pool.tile([128, kin], bf16, tag="bwtb")
    nc.vector.tensor_copy(WTb[0:fout, :], WT[0:fout, :])
    rhs = cp.tile([128, nrhs], f32, tag="rhs" + tag)
    Wn = pool.tile([128, fout], f32, tag="bwn")
    nc.sync.dma_start(Wn[0:kin, :], W_d[:])
    nc.vector.tensor_copy(rhs[:, 0:fout], Wn[:])
    bx = cp.tile([1, nrhs], f32, tag="bx" + tag)
    bn = pool.tile([1, fout], f32, tag="bbn")
    nc.sync.dma_start(bn[:], brow_d[:])
    nc.vector.tensor_copy(bx[:, 0:fout], bn[:])
    bc = pool.tile([128, 1], f32, tag="bbc")
    nc.sync.dma_start(bc[0:fout, :], bcol_d[:])
    for i, A_d in enumerate(A_ds):
        Ab = pool.tile([128, hw], f32, tag="bab")
        nc.sync.dma_start(Ab[0:fout, :], A_d[:])
        Abb = pool.tile([128, hw], bf16, tag="babb")
        nc.vector.tensor_copy(Abb[0:fout, :], Ab[0:fout, :])
        ps = psum.tile([128, hw], f32, tag="ps")
        nc.tensor.matmul(ps[0:kin, :], WTb[0:fout, 0:kin], Abb[0:fout, :],
                         start=True, stop=True)
        nc.vector.tensor_copy(rhs[:, fout + hw * i:fout + hw * (i + 1)],
                              ps[0:kin, :])
        psb = psum.tile([1, hw], f32, tag="ps")
        nc.tensor.matmul(psb[:], bc[0:fout, 0:1], Ab[0:fout, :], start=True,
                         stop=True)
        nc.vector.tensor_copy(bx[:, fout + hw * i:fout + hw * (i + 1)],
                              psb[:])
    rhsb = cp.tile([128, nrhs], bf16, tag="rhsb" + tag)
    nc.vector.tensor_copy(rhsb[:], rhs[:])
    ones = cp.tile([1, 128], f32, tag="ones" + tag)
    nc.gpsimd.memset(ones[:], 1.0)
    bps = psum.tile([128, nrhs], f32, tag="ps")
    nc.tensor.matmul(bps[:], ones[:], bx[:], start=True, stop=True)
    brep = cp.tile([128, nrhs], f32, tag="brep" + tag)
    nc.vector.tensor_copy(brep[:], bps[:])
    return rhs, rhsb, brep, ones


def emit_asb(nc, pool, psum, cp, ones, fs_d, tag):
    fsb = pool.tile([1, 128], f32, tag="fsb")
    nc.sync.dma_start(fsb[:], fs_d[:])
    ps = psum.tile([128, 128], f32, tag="ps")
    nc.tensor.matmul(ps[:], ones[:], fsb[:], start=True, stop=True)
    asb = cp.tile([128, 128], bf16, tag="asb" + tag)
    nc.vector.tensor_copy(asb[:], ps[:])
    return asb


def emit_proj(nc, pool, psum, spool, xN_ds, rhs_list, brep, nrhs, n_tiles,
              F, S, tbl, dst_tbls=None, iotac=None, dS=0):
    """psum = sum_i xN_i_tile^T @ rhs_i, reading node-major tables via
    transpose-DMA; pack [h|scores] rows into tbl and/or
    [dst-scores f32|dmod f32] rows into dst_tbls."""
    so = 64 if F == 128 else 32
    st = [None]
    dstt = [None]
    for c0 in range(0, n_tiles, 8):
        ntc = min(8, n_tiles - c0)
        xbbs = []
        for xd in xN_ds:
            xbb = pool.tile([128, ntc * 128], bf16, tag="pxb")
            nc.sync.dma_start_transpose(
                xbb[:, 0:ntc * 128], xd[c0 * 128:(c0 + ntc) * 128, :])
            xbbs.append(xbb)
        for t in range(ntc):
            gt = c0 + t
            tl = gt % 16
            if tl == 0:
                if tbl is not None:
                    st[0] = spool.tile([128, 16, 128], bf16, tag="stage",
                                       name="stage")
                    nc.gpsimd.memset(st[0][:], 0.0)
                if dst_tbls:
                    dstt[0] = [spool.tile([128, 16, 128], bf16,
                                          tag=f"dst{i}", name=f"dst{i}")
                               for i in range(len(dst_tbls))]
                    for dd in dstt[0]:
                        nc.gpsimd.memset(dd[:], 0.0)
            ps = psum.tile([128, nrhs], f32, tag="ps")
            for i, xbb in enumerate(xbbs):
                nc.tensor.matmul(ps[:], xbb[:, t * 128:(t + 1) * 128],
                                 rhs_list[i][:], start=(i == 0),
                                 stop=(i == len(xbbs) - 1))
            if tbl is not None:
                nc.vector.tensor_tensor(st[0][:, tl, 0:F], ps[:, 0:F],
                                        brep[:, 0:F], op=ADD)
                if S:
                    nc.vector.tensor_tensor(
                        st[0][:].bitcast(f32)[:, tl, so:so + S],
                        ps[:, F:F + S], brep[:, F:F + S], op=ADD)
            if dst_tbls:
                for i in range(len(dst_tbls)):
                    nc.vector.tensor_tensor(
                        dstt[0][i][:].bitcast(f32)[:, tl, 0:dS],
                        ps[:, F + S + dS * i:F + S + dS * (i + 1)],
                        brep[:, F + S + dS * i:F + S + dS * (i + 1)], op=ADD)
                    nc.vector.tensor_copy(
                        dstt[0][i][:].bitcast(f32)[:, tl, dS:dS + 1],
                        iotac[:])
            if tl == 15 or gt == n_tiles - 1:
                cc = gt - tl
                if tbl is not None:
                    t3 = tbl[0:n_tiles * 128, :].rearrange("(c r) e -> r c e",
                                                           r=128)
                    nc.sync.dma_start(t3[:, cc:cc + tl + 1, :],
                                      st[0][:, 0:tl + 1, :])
                if dst_tbls:
                    for i, db in enumerate(dst_tbls):
                        d3 = db[0:n_tiles * 128, :].rearrange(
                            "(c r) e -> r c e", r=128)
                        nc.sync.dma_start(d3[:, cc:cc + tl + 1, :],
                                          dstt[0][i][:, 0:tl + 1, :])


# ----------------------------------------------------------------- kernels --
def build_k1(meta):
    nc = bacc.Bacc(None, target_bir_lowering=False, debug=False,
                   num_swdge_queues=4)
    dt = nc.dram_tensor
    I, O, N = "ExternalInput", "ExternalOutput", "Internal"
    xaL = dt("xaL", [PAD_A, 128], bf16, kind=I)   # own padded author slice
    xpL = dt("xpL", [PAD_P, 128], bf16, kind=I)   # own padded paper slice
    W1a = dt("W1a", [128, 128], f32, kind=I)
    W1aT = dt("W1aT", [128, 128], f32, kind=I)
    W1p = dt("W1p", [128, 128], f32, kind=I)
    W1pT = dt("W1pT", [128, 128], f32, kind=I)
    b1ar = dt("b1ar", [1, 128], f32, kind=I)
    b1ac = dt("b1ac", [128, 1], f32, kind=I)
    b1pr = dt("b1pr", [1, 128], f32, kind=I)
    b1pc = dt("b1pc", [128, 1], f32, kind=I)
    A = {k: dt("A" + k, [128, 8], f32, kind=I)
         for k in ("dap", "dpa", "daa")}
    FS = {k: dt("fs" + k, [1, 128], f32, kind=I)
          for k in ("ap", "pa", "aa")}
    iota_d = dt("iota", [128, 128], f32, kind=I)
    iotac_d = dt("iotac", [128, 1], f32, kind=I)
    zdrow_d = dt("zdrow", [1, 128], bf16, kind=I)
    Wk1 = dt("Wk1", [128, 128], f32, kind=I)
    bk1 = dt("bk1", [128, 1], f32, kind=I)
    q1 = dt("q1", [128, 1], f32, kind=I)
    mio = {}
    for ty in ("ap", "pa", "aa"):
        nt = meta[ty]["n_tiles"]
        mio[ty] = (dt("s16" + ty, [128, nt * 8], i16, kind=I),
                   dt("d16" + ty, [128, nt * 8], i16, kind=I))
    xaB = dt("xaB", [PAD_A, 128], bf16, kind=N)
    xpB = dt("xpB", [PAD_P, 128], bf16, kind=N)
    xa_f = dt("xa_f", [NA_BLK, 128], bf16, kind=N, addr_space="Shared")
    xp_f = dt("xp_f", [NP_BLK, 128], bf16, kind=N, addr_space="Shared")
    au_t = dt("au_t", [NA_BLK, 128], bf16, kind=N)
    pa_t = dt("pa_t", [NP_BLK, 128], bf16, kind=N)
    apd_t = dt("apd_t", [PAD_P + 128, 128], bf16, kind=N)
    pad_t = dt("pad_t", [PAD_A + 128, 128], bf16, kind=N)
    aad_t = dt("aad_t", [PAD_A + 128, 128], bf16, kind=N)
    o_apL = dt("o_apL", [PAD_P, 128], bf16, kind=O)
    o_paL = dt("o_paL", [PAD_A, 128], bf16, kind=O)
    o_aaL = dt("o_aaL", [PAD_A, 128], bf16, kind=O)
    pw1 = dt("pw1", [1, 2], f32, kind=O)

    with tile.TileContext(nc) as tc:
        with (tc.tile_pool(name="c", bufs=1) as cp,
              tc.tile_pool(name="s", bufs=2) as pool,
              tc.tile_pool(name="st", bufs=2) as spool,
              tc.tile_pool(name="a", bufs=1) as apool,
              tc.tile_pool(name="p", bufs=3, space="PSUM") as psum,
              tc.tile_pool(name="p2", bufs=3, space="PSUM") as psum2):
            nc.sync.dma_start(xaB[:], xaL[:])
            nc.sync.dma_start(xpB[:], xpL[:])
            nc.gpsimd.collective_compute("AllGather", BYP, GRP,
                                         ins=[xaB[:]], outs=[xa_f[:]])
            nc.gpsimd.collective_compute("AllGather", BYP, GRP,
                                         ins=[xpB[:]], outs=[xp_f[:]])
            iot = cp.tile([128, 128], f32)
            nc.sync.dma_start(iot[:], iota_d[:])
            iotac = cp.tile([128, 1], f32)
            nc.sync.dma_start(iotac[:], iotac_d[:])
            idf = cp.tile([128, 128], f32)
            nc.vector.tensor_scalar(out=idf[:], in0=iot[:],
                                    scalar1=iotac[:, 0:1], scalar2=None,
                                    op0=EQ)
            zdrow = cp.tile([1, 128], bf16)
            nc.sync.dma_start(zdrow[:], zdrow_d[:])
            for tb, wn in ((apd_t, W_P), (pad_t, W_A), (aad_t, W_A)):
                nc.sync.dma_start(tb[wn * 128:wn * 128 + 1, :], zdrow[:])

            _, ra, bra, ones = build_wa(nc, pool, psum, cp, W1aT, W1a, b1ar,
                                        b1ac, [], 128, 128, 8, "a")
            _, rp, brp, _ = build_wa(nc, pool, psum, cp, W1pT, W1p, b1pr,
                                     b1pc, [], 128, 128, 8, "p")
            _, rpd, brpd, _ = build_wa(nc, pool, psum, cp, W1pT, W1p, b1pr,
                                       b1pc, [A["dap"]], 128, 128, 8, "pd")
            _, rad, brad, _ = build_wa(nc, pool, psum, cp, W1aT, W1a, b1ar,
                                       b1ac, [A["dpa"], A["daa"]],
                                       128, 128, 8, "ad")
            asb_ap = emit_asb(nc, pool, psum, cp, ones, FS["ap"], "ap")
            asb_pa = emit_asb(nc, pool, psum, cp, ones, FS["pa"], "pa")
            asb_aa = emit_asb(nc, pool, psum, cp, ones, FS["aa"], "aa")

            emit_proj(nc, pool, psum, spool, [xa_f], [ra], bra, 128,
                      NA_BLK // 128, 128, 0, au_t)
            emit_proj(nc, pool, psum, spool, [xp_f], [rp], brp, 128,
                      NP_BLK // 128, 128, 0, pa_t)
            emit_proj(nc, pool, psum, spool, [xpL], [rpd], brpd, 136, W_P,
                      128, 0, None, dst_tbls=[apd_t], iotac=iotac, dS=8)
            emit_proj(nc, pool, psum, spool, [xaL], [rad], brad, 144, W_A,
                      128, 0, None, dst_tbls=[pad_t, aad_t],
                      iotac=iotac, dS=8)

            acc = apool.tile([128, W_P * 136], f32, tag="acc")
            nc.gpsimd.memset(acc[:], 0.0)
            emit_edge_phase(nc, pool, psum2, au_t, apd_t, *mio["ap"],
                            meta["ap"], 128, 8, acc, [0, CHK], iot,
                            asb=asb_ap)
            emit_normalize(nc, pool, acc, W_P, 128, 8, o_apL[:], bf16)
            acc = apool.tile([128, W_A * 136], f32, tag="acc")
            nc.gpsimd.memset(acc[:], 0.0)
            emit_edge_phase(nc, pool, psum2, pa_t, pad_t, *mio["pa"],
                            meta["pa"], 128, 8, acc,
                            [0, CHK, 2 * CHK, 3 * CHK], iot, asb=asb_pa)
            emit_normalize(nc, pool, acc, W_A, 128, 8, o_paL[:], bf16)
            acc = apool.tile([128, W_A * 136], f32, tag="acc")
            nc.gpsimd.memset(acc[:], 0.0)
            emit_edge_phase(nc, pool, psum2, au_t, aad_t, *mio["aa"],
                            meta["aa"], 128, 8, acc, [0, CHK], iot,
                            asb=asb_aa)
            emit_normalize(nc, pool, acc, W_A, 128, 8, o_aaL[:], bf16)

            pw = emit_tanh_partial(nc, pool, psum2,
                                   [(o_paL, PAD_A, False),
                                    (o_aaL, PAD_A, False)],
                                   Wk1, bk1, q1, 128, idf, PAD_A - SL_A)
            nc.sync.dma_start(pw1[:], pw[:])
    nc.compile()
    return nc


def build_k2(meta):
    nc = bacc.Bacc(None, target_bir_lowering=False, debug=False,
                   num_swdge_queues=4)
    dt = nc.dram_tensor
    I, O, N = "ExternalInput", "ExternalOutput", "Internal"
    o_apL = dt("o_apL", [PAD_P, 128], bf16, kind=I)
    o_paL = dt("o_paL", [PAD_A, 128], bf16, kind=I)
    o_aaL = dt("o_aaL", [PAD_A, 128], bf16, kind=I)
    pw1 = dt("pw1", [1, 2], f32, kind=I)
    W2a = dt("W2a", [128, 64], f32, kind=I)
    W2aT = dt("W2aT", [64, 128], f32, kind=I)
    W2p = dt("W2p", [128, 64], f32, kind=I)
    W2pT = dt("W2pT", [64, 128], f32, kind=I)
    b2ar = dt("b2ar", [1, 64], f32, kind=I)
    b2ac = dt("b2ac", [64, 1], f32, kind=I)
    b2pr = dt("b2pr", [1, 64], f32, kind=I)
    b2pc = dt("b2pc", [64, 1], f32, kind=I)
    A2 = {k: dt("A2" + k, [64, 4], f32, kind=I)
          for k in ("spa", "dpa", "saa", "daa")}
    iota_d = dt("iota", [128, 128], f32, kind=I)
    iotac_d = dt("iotac", [128, 1], f32, kind=I)
    zdrow_d = dt("zdrow", [1, 128], bf16, kind=I)
    Wk2 = dt("Wk2", [64, 64], f32, kind=I)
    bk2 = dt("bk2", [64, 1], f32, kind=I)
    q2 = dt("q2", [64, 1], f32, kind=I)
    mio = {}
    for ty in ("pa", "aa"):
        nt = meta[ty]["n_tiles"]
        mio[ty] = (dt("s16" + ty, [128, nt * 8], i16, kind=I),
                   dt("d16" + ty, [128, nt * 8], i16, kind=I))
    oapB = dt("oapB", [PAD_P, 128], bf16, kind=N)
    opaB = dt("opaB", [PAD_A, 128], bf16, kind=N)
    oaaB = dt("oaaB", [PAD_A, 128], bf16, kind=N)
    pw1B = dt("pw1B", [1, 2], f32, kind=N)
    oap_f = dt("oap_f", [NP_BLK, 128], bf16, kind=N, addr_space="Shared")
    opa_f = dt("opa_f", [NA_BLK, 128], bf16, kind=N, addr_space="Shared")
    oaa_f = dt("oaa_f", [NA_BLK, 128], bf16, kind=N, addr_space="Shared")
    pw1g = dt("pw1g", [NC, 2], f32, kind=N, addr_space="Shared")
    pw2b = dt("pw2b", [1, 2], f32, kind=N)
    pw2s = dt("pw2s", [1, 2], f32, kind=N, addr_space="Shared")
    au_t = dt("au_t", [NA_BLK, 128], bf16, kind=N)
    pa_t = dt("pa_t", [NP_BLK, 128], bf16, kind=N)
    pad_t = dt("pad_t", [PAD_A + 128, 128], bf16, kind=N)
    aad_t = dt("aad_t", [PAD_A + 128, 128], bf16, kind=N)
    o2pa = dt("o2pa", [PAD_A, 64], f32, kind=N)
    o2aa = dt("o2aa", [PAD_A, 64], f32, kind=N)
    out = dt("out", [PAD_A, 64], f32, kind=O)

    with tile.TileContext(nc) as tc:
        with (tc.tile_pool(name="c", bufs=1) as cp,
              tc.tile_pool(name="s", bufs=2) as pool,
              tc.tile_pool(name="st", bufs=2) as spool,
              tc.tile_pool(name="a", bufs=1) as apool,
              tc.tile_pool(name="p", bufs=3, space="PSUM") as psum,
              tc.tile_pool(name="p2", bufs=3, space="PSUM") as psum2):
            nc.sync.dma_start(oapB[:], o_apL[:])
            nc.sync.dma_start(opaB[:], o_paL[:])
            nc.sync.dma_start(oaaB[:], o_aaL[:])
            nc.sync.dma_start(pw1B[:], pw1[:])
            nc.gpsimd.collective_compute("AllGather", BYP, GRP,
                                         ins=[oapB[:]], outs=[oap_f[:]])
            nc.gpsimd.collective_compute("AllGather", BYP, GRP,
                                         ins=[opaB[:]], outs=[opa_f[:]])
            nc.gpsimd.collective_compute("AllGather", BYP, GRP,
                                         ins=[oaaB[:]], outs=[oaa_f[:]])
            nc.gpsimd.collective_compute("AllGather", BYP, GRP,
                                         ins=[pw1B[:]], outs=[pw1g[:]])
            iot = cp.tile([128, 128], f32)
            nc.sync.dma_start(iot[:], iota_d[:])
            iotac = cp.tile([128, 1], f32)
            nc.sync.dma_start(iotac[:], iotac_d[:])
            idf = cp.tile([128, 128], f32)
            nc.vector.tensor_scalar(out=idf[:], in0=iot[:],
                                    scalar1=iotac[:, 0:1], scalar2=None,
                                    op0=EQ)
            zdrow = cp.tile([1, 128], bf16)
            nc.sync.dma_start(zdrow[:], zdrow_d[:])
            for tb in (pad_t, aad_t):
                nc.sync.dma_start(tb[W_A * 128:W_A * 128 + 1, :], zdrow[:])

            ra_f, _, bra, ones = build_wa(nc, pool, psum, cp, W2aT, W2a,
                                          b2ar, b2ac, [A2["saa"]],
                                          128, 64, 4, "a")
            _, rp, brp, _ = build_wa(nc, pool, psum, cp, W2pT, W2p, b2pr,
                                     b2pc, [A2["spa"]], 128, 64, 4, "p")
            rad_f, _, brad, _ = build_wa(nc, pool, psum, cp, W2aT, W2a, b2ar,
                                         b2ac, [A2["dpa"], A2["daa"]],
                                         128, 64, 4, "ad")
            # beta1 from gathered pw1: s = sum_c pw1g[c, :] / N_A
            pwt = pool.tile([8, 2], f32, tag="pwt")
            nc.sync.dma_start(pwt[:], pw1g[:])
            ones8 = cp.tile([8, 1], f32, tag="ones8")
            nc.gpsimd.memset(ones8[:], 1.0)
            ps_s = psum.tile([1, 2], f32, tag="ps")
            nc.tensor.matmul(ps_s[:], ones8[:], pwt[:], start=True,
                             stop=True)
            s1 = pool.tile([1, 2], f32, tag="s1")
            nc.vector.tensor_scalar(out=s1[:], in0=ps_s[:],
                                    scalar1=1.0 / N_A, scalar2=None,
                                    op0=MULT)
            bcols = emit_beta_tail(nc, pool, psum, s1, ones)
            ra_s, rad_s = [], []
            for m in range(2):
                rs = cp.tile([128, 68], bf16, tag=f"ras{m}")
                nc.scalar.activation(rs[:], ra_f[:], AF.Copy,
                                     scale=bcols[m][:])
                ra_s.append(rs)
                rds = cp.tile([128, 72], bf16, tag=f"rads{m}")
                nc.scalar.activation(rds[:], rad_f[:], AF.Copy,
                                     scale=bcols[m][:])
                rad_s.append(rds)

            emit_proj(nc, pool, psum, spool, [opa_f, oaa_f], ra_s, bra, 68,
                      NA_BLK // 128, 64, 4, au_t)
            emit_proj(nc, pool, psum, spool, [oap_f], [rp], brp, 68,
                      NP_BLK // 128, 64, 4, pa_t)
            emit_proj(nc, pool, psum, spool, [o_paL, o_aaL], rad_s, brad, 72,
                      W_A, 64, 0, None, dst_tbls=[pad_t, aad_t],
                      iotac=iotac, dS=4)

            acc = apool.tile([128, W_A * 68], f32, tag="acc")
            nc.gpsimd.memset(acc[:], 0.0)
            emit_edge_phase(nc, pool, psum2, pa_t, pad_t, *mio["pa"],
                            meta["pa"], 64, 4, acc,
                            [0, CHK, 2 * CHK, 3 * CHK], iot, so=32)
            emit_normalize(nc, pool, acc, W_A, 64, 4, o2pa[:], f32)
            acc = apool.tile([128, W_A * 68], f32, tag="acc")
            nc.gpsimd.memset(acc[:], 0.0)
            emit_edge_phase(nc, pool, psum2, au_t, aad_t, *mio["aa"],
                            meta["aa"], 64, 4, acc, [0, CHK], iot, so=32)
            emit_normalize(nc, pool, acc, W_A, 64, 4, o2aa[:], f32)

            pw = emit_tanh_partial(nc, pool, psum2,
                                   [(o2pa, PAD_A, True), (o2aa, PAD_A, True)],
                                   Wk2, bk2, q2, 64, idf, PAD_A - SL_A)
            nc.gpsimd.dma_start(pw2b[:], pw[:])
            nc.gpsimd.collective_compute("AllReduce", ADD, GRP,
                                         ins=[pw2b[:]], outs=[pw2s[:]])
            pw2t = pool.tile([1, 2], f32, tag="pw2t")
            nc.sync.dma_start(pw2t[:], pw2s[:])
            s2 = pool.tile([1, 2], f32, tag="s2")
            nc.vector.tensor_scalar(out=s2[:], in0=pw2t[:],
                                    scalar1=1.0 / N_A, scalar2=None,
                                    op0=MULT)
            bcols2 = emit_beta_tail(nc, pool, psum, s2, ones)
            for t in range(PAD_A // 128):
                a = pool.tile([128, 64], f32, tag="ta")
                b = pool.tile([128, 64], f32, tag="tb")
                nc.sync.dma_start(a[:], o2pa[t * 128:(t + 1) * 128, :])
                nc.sync.dma_start(b[:], o2aa[t * 128:(t + 1) * 128, :])
                nc.vector.tensor_scalar(out=a[:], in0=a[:],
                                        scalar1=bcols2[0][:, 0:1],
                                        scalar2=None, op0=MULT)
                nc.vector.tensor_scalar(out=b[:], in0=b[:],
                                        scalar1=bcols2[1][:, 0:1],
                                        scalar2=None, op0=MULT)
                nc.vector.tensor_tensor(a[:], a[:], b[:], op=ADD)
                nc.sync.dma_start(out[t * 128:(t + 1) * 128, :], a[:])
    nc.compile()
    return nc


# ------------------------------------------------------------------ driver --
DBG = {}
EXEC_NS = []


def _make_runner(nc, mesh, sh):
    """jit-of-shard_map wrapper for one Bass program (axon bass_exec path)."""
    import jax
    from jax.sharding import PartitionSpec
    from concourse.bass2jax import (_bass_exec_p, partition_id_tensor,
                                    install_neuronx_cc_hook)
    install_neuronx_cc_hook()
    pname = nc.partition_id_tensor.name if nc.partition_id_tensor else None
    in_names, out_names, out_avals, zero_shapes = [], [], [], []
    for alloc in nc.m.functions[0].allocations:
        if not isinstance(alloc, mybir.MemoryLocationSet):
            continue
        name = alloc.memorylocations[0].name
        if alloc.kind == "ExternalInput":
            if name != pname:
                in_names.append(name)
        elif alloc.kind == "ExternalOutput":
            out_names.append(name)
            shape = tuple(alloc.tensor_shape)
            dtype = mybir.dt.np(alloc.dtype)
            out_avals.append(jax.core.ShapedArray(shape, dtype))
            zero_shapes.append((shape, dtype))
    n_params, n_outs = len(in_names), len(out_avals)
    all_in = list(in_names) + list(out_names)
    if pname is not None:
        all_in.append(pname)

    def _body(*args):
        operands = list(args)
        if pname is not None:
            operands.append(partition_id_tensor())
        return tuple(_bass_exec_p.bind(
            *operands, out_avals=tuple(out_avals), in_names=tuple(all_in),
            out_names=tuple(out_names), lowering_input_output_aliases=(),
            sim_require_finite=True, sim_require_nnan=True, nc=nc))

    spec = PartitionSpec("core")
    # no donation: every ExternalOutput is fully written, so the zero
    # "output seed" operands are dead inputs reusable across repetitions.
    fn = jax.jit(
        jax.shard_map(_body, mesh=mesh, in_specs=(spec,) * (n_params + n_outs),
                      out_specs=(spec,) * n_outs, check_vma=False),
        keep_unused=True)
    return fn, in_names, out_names, zero_shapes


def kernel(**inp):
    import time
    import jax
    from jax.sharding import Mesh, PartitionSpec, NamedSharding

    inp = {k: np.asarray(v) for k, v in inp.items()}
    m1 = {"ap": prep_type(inp["ei_ap_src"], inp["ei_ap_dst"], N_P,
                          SL_A, PAD_A, W_P),
          "pa": prep_type(inp["ei_pa_src"], inp["ei_pa_dst"], N_A,
                          SL_P, PAD_P, W_A),
          "aa": prep_type(inp["ei_aa_src"], inp["ei_aa_dst"], N_A,
                          SL_A, PAD_A, W_A)}
    iota = np.tile(np.arange(128, dtype=np.float32), (128, 1))
    iotac = np.arange(128, dtype=np.float32)[:, None]
    zdrow_f = np.zeros(64, np.float32)
    zdrow_f[4] = -1.0   # layer-2 dmod slot
    zdrow_f[8] = -1.0   # layer-1 dmod slot
    zdrow = zdrow_f.view(BF)[None, :]

    devices = jax.devices()[:NC]
    mesh = Mesh(np.asarray(devices), ("core",))
    P = PartitionSpec
    sh = NamedSharding(mesh, P("core"))

    nc1 = build_k1(m1)
    nc2 = build_k2({"pa": m1["pa"], "aa": m1["aa"]})
    f1, in1_names, out1_names, zs1 = _make_runner(nc1, mesh, sh)
    f2, in2_names, out2_names, zs2 = _make_runner(nc2, mesh, sh)

    def put_repl(x):
        x = np.asarray(x)
        shards = [jax.device_put(x, d) for d in devices]
        gshape = (NC * x.shape[0],) + x.shape[1:]
        return jax.make_array_from_single_device_arrays(gshape, sh, shards)

    def put_percore(xs):
        xs = [np.asarray(x) for x in xs]
        shards = [jax.device_put(x, d) for x, d in zip(xs, devices)]
        gshape = (NC * xs[0].shape[0],) + xs[0].shape[1:]
        return jax.make_array_from_single_device_arrays(gshape, sh, shards)

    const = dict(
        W1a=inp["W1_a"], W1aT=np.ascontiguousarray(inp["W1_a"].T),
        W1p=inp["W1_p"], W1pT=np.ascontiguousarray(inp["W1_p"].T),
        b1ar=inp["b1_a"][None, :], b1ac=inp["b1_a"][:, None],
        b1pr=inp["b1_p"][None, :], b1pc=inp["b1_p"][:, None],
        Adap=ablk(inp["a1d_ap"], 128), Adpa=ablk(inp["a1d_pa"], 128),
        Adaa=ablk(inp["a1d_aa"], 128),
        fsap=inp["a1s_ap"].reshape(1, 128),
        fspa=inp["a1s_pa"].reshape(1, 128),
        fsaa=inp["a1s_aa"].reshape(1, 128),
        iota=iota, iotac=iotac, zdrow=zdrow,
        Wk1=inp["Wk1"], bk1=inp["bk1"][:, None], q1=inp["q1"][:, None],
        W2a=inp["W2_a"], W2aT=np.ascontiguousarray(inp["W2_a"].T),
        W2p=inp["W2_p"], W2pT=np.ascontiguousarray(inp["W2_p"].T),
        b2ar=inp["b2_a"][None, :], b2ac=inp["b2_a"][:, None],
        b2pr=inp["b2_p"][None, :], b2pc=inp["b2_p"][:, None],
        A2spa=ablk(inp["a2s_pa"], 64), A2dpa=ablk(inp["a2d_pa"], 64),
        A2saa=ablk(inp["a2s_aa"], 64), A2daa=ablk(inp["a2d_aa"], 64),
        Wk2=inp["Wk2"], bk2=inp["bk2"][:, None], q2=inp["q2"][:, None])
    dev = {k: put_repl(v) for k, v in const.items()}
    for ty in ("ap", "pa", "aa"):
        dev["s16" + ty] = put_percore(m1[ty]["s16"])
        dev["d16" + ty] = put_percore(m1[ty]["d16"])
    xa = inp["x_author"].astype(BF)
    xp = inp["x_paper"].astype(BF)
    dev["xaL"] = put_percore([
        np.pad(xa[c * SL_A:(c + 1) * SL_A], ((0, PAD_A - SL_A), (0, 0)))
        for c in range(NC)])
    dev["xpL"] = put_percore([
        np.pad(xp[c * SL_P:(c + 1) * SL_P], ((0, PAD_P - SL_P), (0, 0)))
        for c in range(NC)])
    jax.block_until_ready(list(dev.values()))

    def zeros(zshapes):
        z = [put_percore([np.zeros(s, d)] * NC) for s, d in zshapes]
        jax.block_until_ready(z)
        return z

    z1, z2 = zeros(zs1), zeros(zs2)

    def chain():
        o1 = f1(*[dev[n] for n in in1_names], *z1)
        o1m = dict(zip(out1_names, o1))
        io2 = dict(dev, **o1m)
        o2 = f2(*[io2[n] for n in in2_names], *z2)
        return o2[out2_names.index("out")]

    out_d = chain()
    jax.block_until_ready(out_d)

    # timed: N complete model executions enqueued back-to-back, one sync
    N = 64
    t0 = time.time()
    outs = [chain() for _ in range(N)]
    out_d = outs[-1]
    jax.block_until_ready(outs)
    EXEC_NS.append(int((time.time() - t0) * 1e9) // N)

    out_g = np.asarray(out_d)
    out = np.zeros((N_A, 64), np.float32)
    for c in range(NC):
        out[c * SL_A:(c + 1) * SL_A] = out_g[c * PAD_A:c * PAD_A + SL_A]
    return out
